# revision 68
# baseline (speedup 1.0000x reference)
"""CGC (Customized Gate Control) MoE layer on 8 Trainium2 NeuronCores.

Strategy: data-parallel over batch. B=4096 is split into 8 shards of 512
rows; every core holds all 8 expert MLPs (weights replicated in its
in_map) and computes the full layer for its shard — no collectives.

Per-core dataflow (BL=512 local batch):
  - x inputs are PE-transposed into xT [D-part, B-free] layout.
  - Expert layer 1: hT[H1,B] = relu(W1.T-free matmul) with per-partition
    bias fused into the ScalarE activation.
  - Expert layer 2: out[B,H2] natural layout; b2 is broadcast to a
    [128,H2] tile once per expert (rank-1 matmul ones.T @ b2), then the
    bias-add + relu run on VectorE (add + max). The final expert instead
    appends the rank-1 matmul to its PSUM group and relus on ScalarE,
    which shortens the kernel-tail dependency chain.
  - Gates: logits via matmul (lhsT=Wg, moving xT), bias on ScalarE,
    PE-transpose back to [B-part, K], softmax along the free dim.
  - Gated combine: single-instruction MAC on VectorE
    (scalar_tensor_tensor: acc = oe * gw[:,col] + acc).
  - x for the next domain is prefetched during the current domain's
    first expert; W2 loads are split into 512KB slabs and deferred past
    the W1 slabs they'd otherwise delay.
Matmuls run as float32r (full-rate fp32 at moving-dim >= 256, ~fp32
accuracy: 2.3e-4 max rel err vs the fp32 reference on hardware).
"""

import numpy as np

import concourse.tile as tile
from concourse import bacc, mybir
from concourse.bass_utils import run_bass_kernel_spmd

N_CORES = 8
B = 4096
BL = B // N_CORES  # 512 rows per core
D = 1024
H1 = 1024
H2 = 512
DOM = 3
NES = 2
NSH = 2
E_SPEC = DOM * NES  # 6
GATE_K = NES + NSH  # 4
TOTAL_E = E_SPEC + NSH  # 8

F32 = mybir.dt.float32
F32R = mybir.dt.float32r
AX = mybir.AxisListType
AF = mybir.ActivationFunctionType
ALU = mybir.AluOpType

NBT = BL // 128  # 4 batch tiles per core
NKD = D // 128   # 8 contraction tiles over D
NKH = H1 // 128  # 8 contraction tiles over H1
NMH = H1 // 128  # 8 output tiles over H1


def _build_nc(mm_dt=F32R):
    from contextlib import ExitStack

    nc = bacc.Bacc("TRN2", target_bir_lowering=False, debug=False)

    xs = [
        nc.dram_tensor(n, [BL, D], F32, kind="ExternalInput")
        for n in ("x0", "x1", "x2", "x_shared")
    ]
    W1s = nc.dram_tensor("W1s", [E_SPEC, D, H1], mm_dt, kind="ExternalInput")
    b1s = nc.dram_tensor("b1s", [E_SPEC, H1], F32, kind="ExternalInput")
    W2s = nc.dram_tensor("W2s", [E_SPEC, H1, H2], mm_dt, kind="ExternalInput")
    b2s = nc.dram_tensor("b2s", [E_SPEC, H2], mm_dt, kind="ExternalInput")
    W1h = nc.dram_tensor("W1h", [NSH, D, H1], mm_dt, kind="ExternalInput")
    b1h = nc.dram_tensor("b1h", [NSH, H1], F32, kind="ExternalInput")
    W2h = nc.dram_tensor("W2h", [NSH, H1, H2], mm_dt, kind="ExternalInput")
    b2h = nc.dram_tensor("b2h", [NSH, H2], mm_dt, kind="ExternalInput")
    Wg = nc.dram_tensor("Wg", [DOM, D, GATE_K], mm_dt, kind="ExternalInput")
    bg = nc.dram_tensor("bg", [DOM, GATE_K], F32, kind="ExternalInput")
    Wsg = nc.dram_tensor("Wsg", [D, TOTAL_E], mm_dt, kind="ExternalInput")
    bsg = nc.dram_tensor("bsg", [TOTAL_E], F32, kind="ExternalInput")
    ys = [
        nc.dram_tensor(n, [BL, H2], F32, kind="ExternalOutput")
        for n in ("y0", "y1", "y2", "ysh")
    ]

    ident_dram = nc.inline_tensor(np.eye(128, dtype=np.float32), name="ident")
    ones_dram = nc.inline_tensor(np.ones((1, 128), np.float32), name="ones1")

    with tile.TileContext(nc) as tc, ExitStack() as ctx:
        p_const = ctx.enter_context(tc.tile_pool(name="const", bufs=1))
        p_xstage = ctx.enter_context(tc.tile_pool(name="xstage", bufs=2))
        p_xT = ctx.enter_context(tc.tile_pool(name="xT", bufs=2))
        p_w1 = ctx.enter_context(tc.tile_pool(name="w1", bufs=4))
        p_w2 = ctx.enter_context(tc.tile_pool(name="w2", bufs=2))
        p_h = ctx.enter_context(tc.tile_pool(name="hT", bufs=2))
        p_oe = ctx.enter_context(tc.tile_pool(name="oe", bufs=2))
        p_osh = ctx.enter_context(tc.tile_pool(name="osh", bufs=1))
        p_acc = ctx.enter_context(tc.tile_pool(name="acc", bufs=1))
        p_bias = ctx.enter_context(tc.tile_pool(name="bias", bufs=2))
        p_gw = ctx.enter_context(tc.tile_pool(name="gw", bufs=1))
        p_gt = ctx.enter_context(tc.tile_pool(name="gt", bufs=2))
        p_sm = ctx.enter_context(tc.tile_pool(name="sm", bufs=3))
        p_tmp = ctx.enter_context(tc.tile_pool(name="tmp", bufs=2))
        ps_h = ctx.enter_context(tc.tile_pool(name="psh", bufs=3, space="PSUM"))
        ps_o = ctx.enter_context(tc.tile_pool(name="pso", bufs=2, space="PSUM"))
        ps_t = ctx.enter_context(tc.tile_pool(name="pst", bufs=3, space="PSUM"))

        identr_sb = p_const.tile([128, 128], mm_dt)
        nc.sync.dma_start(out=identr_sb, in_=ident_dram[:].bitcast(mm_dt))
        ident_sb = p_const.tile([128, 128], F32)
        nc.sync.dma_start(out=ident_sb, in_=ident_dram[:])
        # PE warm-up: harmless matmuls on the identity while the first x/W
        # DMAs are in flight, so the HAM clock gate opens before real work.
        for _ in range(24):
            pw = ps_t.tile([128, 128], F32, tag="pt", name="pw")
            nc.tensor.matmul(pw, lhsT=identr_sb, rhs=identr_sb, start=True, stop=True)
        def transpose_x(x_dram):
            """[BL, D] natural -> xT tile [128, NKD, BL] (d on partitions).

            j-outer so slab j is complete (and consumable by L1/gates)
            after only NBT transposes; 4 transposes share one PSUM bank and
            drain with a single contiguous ACT copy.
            """
            xT = p_xT.tile([128, NKD, BL], mm_dt, tag="xT")
            xsts = x_dram
            for j in range(NKD):
                pt = ps_t.tile([128, BL], mm_dt, tag="pt")
                for bt in range(NBT):
                    nc.tensor.transpose(
                        pt[:, bt * 128 : (bt + 1) * 128],
                        xsts[bt][:, j * 128 : (j + 1) * 128],
                        identr_sb,
                    )
                nc.scalar.copy(out=xT[:, j, :], in_=pt)
            return xT

        def load_xstage(x_dram, bts=range(NBT)):
            xsts = []
            for bt in bts:
                xst = p_xstage.tile(
                    [128, D], mm_dt, tag=f"xst{bt}", name=f"xst{bt}", bufs=1
                )
                nc.sync.dma_start(
                    out=xst,
                    in_=x_dram[bt * 128 : (bt + 1) * 128, :].bitcast(mm_dt),
                )
                xsts.append(xst)
            return xsts

        def compute_gate(xT, wg_2d, bias_1d, K, tag):
            """softmax(x @ Wg + bg) -> gw tile [128, NBT, K] (b on partitions)."""
            wg_sb = p_sm.tile([128, NKD, K], mm_dt, tag=f"wg{K}")
            nc.sync.dma_start(
                out=wg_sb, in_=wg_2d.rearrange("(kt p) k -> p kt k", p=128)
            )
            bg_sb = p_sm.tile([K, 1], F32, tag=f"bg{K}")
            nc.sync.dma_start(
                out=bg_sb, in_=bias_1d.rearrange("(k one) -> k one", one=1)
            )
            pg = ps_t.tile([K, BL], F32, tag="pt")
            for kt in range(NKD):
                nc.tensor.matmul(
                    pg,
                    lhsT=wg_sb[:, kt, :],
                    rhs=xT[:, kt, :],
                    start=(kt == 0),
                    stop=(kt == NKD - 1),
                )
            glT = p_gt.tile([K, BL], F32, tag="glT")
            nc.scalar.activation(
                out=glT, in_=pg, func=AF.Identity, bias=bg_sb, scale=1.0
            )
            gw = p_gw.tile([128, NBT, K], F32, tag=tag)
            for bt in range(NBT):
                ptg = ps_t.tile([128, K], F32, tag="pt")
                nc.tensor.transpose(
                    ptg, glT[:, bt * 128 : (bt + 1) * 128], ident_sb[:K, :K]
                )
                nm = p_sm.tile([128, 1], F32, tag="nm")
                nc.vector.reduce_max(out=nm, in_=ptg, axis=AX.X, negate=True)
                esb = p_sm.tile([128, K], F32, tag="esb")
                nc.scalar.activation(
                    out=esb, in_=ptg, func=AF.Exp, bias=nm, scale=1.0
                )
                ssb = p_sm.tile([128, 1], F32, tag="ssb")
                nc.vector.reduce_sum(out=ssb, in_=esb, axis=AX.X)
                rsb = p_sm.tile([128, 1], F32, tag="rsb")
                nc.vector.reciprocal(out=rsb, in_=ssb)
                nc.vector.tensor_scalar_mul(gw[:, bt, :], esb, rsb)
            return gw

        def expert(xT, w1_2d, b1_1d, w2_2d, b2_1d, out_pool, tag, bias_mm=False):
            """Two-layer MLP: relu(relu(x@W1+b1)@W2+b2) -> [128, NBT, H2]."""
            b1_sb = p_bias.tile([128, NMH], F32, tag="b1")
            nc.sync.dma_start(
                out=b1_sb, in_=b1_1d.rearrange("(mt p) -> p mt", p=128)
            )
            b2_sb = p_bias.tile([1, H2], mm_dt, tag="b2")
            nc.sync.dma_start(
                out=b2_sb, in_=b2_1d.rearrange("(one o) -> one o", one=1)
            )
            if not bias_mm:
                bb = ps_t.tile([128, H2], F32, tag="pt", name="bb")
                nc.tensor.matmul(
                    bb, lhsT=ones_sb, rhs=b2_sb, start=True, stop=True
                )
                b2bc = p_tmp.tile([128, H2], F32, tag="b2bc", name="b2bc")
                nc.scalar.copy(out=b2bc, in_=bb)
            w1r = w1_2d.rearrange("(kt p) h -> p kt h", p=128)
            hT = p_h.tile([128, NMH, BL], mm_dt, tag="hT")
            for mt in range(NMH):
                w1_sb = p_w1.tile([128, NKD, 128], mm_dt, tag="w1")
                nc.sync.dma_start(
                    out=w1_sb, in_=w1r[:, :, mt * 128 : (mt + 1) * 128]
                )

                ph = ps_h.tile([128, BL], F32, tag="ph")
                for kt in range(NKD):
                    nc.tensor.matmul(
                        ph,
                        lhsT=w1_sb[:, kt, :],
                        rhs=xT[:, kt, :],
                        start=(kt == 0),
                        stop=(kt == NKD - 1),
                    )
                nc.scalar.activation(
                    out=hT[:, mt, :],
                    in_=ph,
                    func=AF.Relu,
                    bias=b1_sb[:, mt : mt + 1],
                    scale=1.0,
                )
            w2_sb = p_w2.tile([128, NKH, H2], mm_dt, tag="w2")
            w2r = w2_2d.rearrange("(kt p) o -> p kt o", p=128)
            for g in range(0, NKH, 2):
                nc.sync.dma_start(
                    out=w2_sb[:, g : g + 2, :], in_=w2r[:, g : g + 2, :]
                )
            oe = out_pool.tile([128, NBT, H2], F32, tag=tag)
            for bt in range(NBT):
                po = ps_o.tile([128, H2], F32, tag="po")
                po2 = po
                for kt in range(NKH):
                    nc.tensor.matmul(
                        po,
                        lhsT=hT[:, kt, bt * 128 : (bt + 1) * 128],
                        rhs=w2_sb[:, kt, :],
                        start=(kt == 0),
                        stop=(False if bias_mm else kt == NKH - 1),
                    )
                if bias_mm:
                    nc.tensor.matmul(
                        po2, lhsT=ones_sb, rhs=b2_sb, start=False, stop=True
                    )
                    nc.scalar.activation(out=oe[:, bt, :], in_=po2, func=AF.Relu)
                else:
                    nc.vector.tensor_tensor(oe[:, bt, :], po, b2bc, ALU.add)
                    nc.gpsimd.tensor_scalar_max(oe[:, bt, :], oe[:, bt, :], 0.0)
            return oe

        accs = [None] * 4

        def accumulate(acc_idx, oe, gw, col, first):
            acc = accs[acc_idx]
            for bt in range(NBT):
                if first:
                    nc.vector.tensor_scalar_mul(
                        acc[:, bt, :], oe[:, bt, :], gw[:, bt, col : col + 1]
                    )
                else:
                    nc.vector.scalar_tensor_tensor(
                        out=acc[:, bt, :],
                        in0=oe[:, bt, :],
                        scalar=gw[:, bt, col : col + 1],
                        in1=acc[:, bt, :],
                        op0=ALU.mult,
                        op1=ALU.add,
                    )

        # ---- shared phase: shared experts kept resident, shared gate ----
        xT_sh = transpose_x(load_xstage(xs[3]))
        gws = compute_gate(xT_sh, Wsg[:], bsg[:], TOTAL_E, tag="gws")
        ones_sb = p_const.tile([1, 128], mm_dt)
        nc.sync.dma_start(out=ones_sb, in_=ones_dram[:].bitcast(mm_dt))
        osh = []
        xsts_next = None
        for j in range(NSH):
            o = expert(
                xT_sh, W1h[j], b1h[j], W2h[j], b2h[j], p_osh, tag=f"osh{j}"
            )
            osh.append(o)
            if j == 0:
                xsts_next = load_xstage(xs[0])
        accs[3] = p_acc.tile([128, NBT, H2], F32, tag="acc3", name="acc3")
        accumulate(3, osh[0], gws, E_SPEC + 0, first=True)
        accumulate(3, osh[1], gws, E_SPEC + 1, first=False)

        # ---- domain phases ----
        for d in range(DOM):
            xT_d = transpose_x(xsts_next)
            gw_d = compute_gate(xT_d, Wg[d], bg[d], GATE_K, tag=f"gw{d}")
            accs[d] = p_acc.tile(
                [128, NBT, H2], F32, tag=f"acc{d}", name=f"acc{d}"
            )
            accumulate(d, osh[0], gw_d, NES + 0, first=True)
            accumulate(d, osh[1], gw_d, NES + 1, first=False)
            for i in range(NES):
                e = d * NES + i
                oe = expert(
                    xT_d, W1s[e], b1s[e], W2s[e], b2s[e], p_oe, tag="oe",
                    bias_mm=(e == E_SPEC - 1),
                )
                if i == 0 and d < DOM - 1:
                    xsts_next = load_xstage(xs[d + 1])
                accumulate(d, oe, gw_d, i, first=False)
                accumulate(3, oe, gws, e, first=False)
            yr = ys[d][:].rearrange("(bt p) o -> bt p o", p=128)
            for bt in range(NBT):
                nc.sync.dma_start(out=yr[bt], in_=accs[d][:, bt, :])
        yr3 = ys[3][:].rearrange("(bt p) o -> bt p o", p=128)
        for bt in range(NBT):
            nc.sync.dma_start(out=yr3[bt], in_=accs[3][:, bt, :])

    nc.compile()
    return nc


_NC_CACHE = {}


def _get_nc(mm_dt=F32R):
    key = str(mm_dt)
    if key not in _NC_CACHE:
        _NC_CACHE[key] = _build_nc(mm_dt)
    return _NC_CACHE[key]


def kernel(**inputs):
    return run_kernel(inputs)


def run_kernel(inputs, mm_dt=F32R, trace=False):
    nc = _get_nc(mm_dt)
    shard_names = ("x0", "x1", "x2", "x_shared")
    full = {k: np.ascontiguousarray(np.asarray(v, dtype=np.float32)) for k, v in inputs.items()}
    in_maps = []
    for c in range(N_CORES):
        m = {}
        for k, v in full.items():
            if k in shard_names:
                m[k] = v[c * BL : (c + 1) * BL]
            else:
                m[k] = v
        in_maps.append(m)
    res = run_bass_kernel_spmd(nc, in_maps, list(range(N_CORES)), trace=trace)
    outs = []
    for name in ("y0", "y1", "y2", "ysh"):
        outs.append(
            np.concatenate([res.results[c][name] for c in range(N_CORES)], axis=0)
        )
    out = tuple(outs)
    if trace:
        return out, res
    return out


# revision 73
# speedup vs baseline: 1.0028x; 1.0028x over previous
"""CGC (Customized Gate Control) MoE layer on 8 Trainium2 NeuronCores.

Strategy: data-parallel over batch. B=4096 is split into 8 shards of 512
rows; every core holds all 8 expert MLPs (weights replicated in its
in_map) and computes the full layer for its shard — no collectives.

Per-core dataflow (BL=512 local batch):
  - x inputs are PE-transposed into xT [D-part, B-free] layout.
  - Expert layer 1: hT[H1,B] = relu(W1.T-free matmul) with per-partition
    bias fused into the ScalarE activation.
  - Expert layer 2: out[B,H2] natural layout; b2 is broadcast to a
    [128,H2] tile once per expert (rank-1 matmul ones.T @ b2), then the
    bias-add + relu run on VectorE (add + max). The final expert instead
    appends the rank-1 matmul to its PSUM group and relus on ScalarE,
    which shortens the kernel-tail dependency chain.
  - Gates: logits via matmul (lhsT=Wg, moving xT), bias on ScalarE,
    PE-transpose back to [B-part, K], softmax along the free dim.
  - Gated combine: single-instruction MAC on VectorE
    (scalar_tensor_tensor: acc = oe * gw[:,col] + acc).
  - x for the next domain is prefetched during the current domain's
    first expert; W2 loads are split into 512KB slabs and deferred past
    the W1 slabs they'd otherwise delay.
Matmuls run as float32r (full-rate fp32 at moving-dim >= 256, ~fp32
accuracy: 2.3e-4 max rel err vs the fp32 reference on hardware).
"""

import numpy as np

import concourse.tile as tile
from concourse import bacc, mybir
from concourse.bass_utils import run_bass_kernel_spmd

N_CORES = 8
B = 4096
BL = B // N_CORES  # 512 rows per core
D = 1024
H1 = 1024
H2 = 512
DOM = 3
NES = 2
NSH = 2
E_SPEC = DOM * NES  # 6
GATE_K = NES + NSH  # 4
TOTAL_E = E_SPEC + NSH  # 8

F32 = mybir.dt.float32
F32R = mybir.dt.float32r
AX = mybir.AxisListType
AF = mybir.ActivationFunctionType
ALU = mybir.AluOpType

NBT = BL // 128  # 4 batch tiles per core
NKD = D // 128   # 8 contraction tiles over D
NKH = H1 // 128  # 8 contraction tiles over H1
NMH = H1 // 128  # 8 output tiles over H1


def _build_nc(mm_dt=F32R):
    from contextlib import ExitStack

    nc = bacc.Bacc("TRN2", target_bir_lowering=False, debug=False)

    xs = [
        nc.dram_tensor(n, [BL, D], F32, kind="ExternalInput")
        for n in ("x0", "x1", "x2", "x_shared")
    ]
    W1s = nc.dram_tensor("W1s", [E_SPEC, D, H1], mm_dt, kind="ExternalInput")
    b1s = nc.dram_tensor("b1s", [E_SPEC, H1], F32, kind="ExternalInput")
    W2s = nc.dram_tensor("W2s", [E_SPEC, H1, H2], mm_dt, kind="ExternalInput")
    b2s = nc.dram_tensor("b2s", [E_SPEC, H2], mm_dt, kind="ExternalInput")
    W1h = nc.dram_tensor("W1h", [NSH, D, H1], mm_dt, kind="ExternalInput")
    b1h = nc.dram_tensor("b1h", [NSH, H1], F32, kind="ExternalInput")
    W2h = nc.dram_tensor("W2h", [NSH, H1, H2], mm_dt, kind="ExternalInput")
    b2h = nc.dram_tensor("b2h", [NSH, H2], mm_dt, kind="ExternalInput")
    Wg = nc.dram_tensor("Wg", [DOM, D, GATE_K], mm_dt, kind="ExternalInput")
    bg = nc.dram_tensor("bg", [DOM, GATE_K], F32, kind="ExternalInput")
    Wsg = nc.dram_tensor("Wsg", [D, TOTAL_E], mm_dt, kind="ExternalInput")
    bsg = nc.dram_tensor("bsg", [TOTAL_E], F32, kind="ExternalInput")
    ys = [
        nc.dram_tensor(n, [BL, H2], F32, kind="ExternalOutput")
        for n in ("y0", "y1", "y2", "ysh")
    ]


    with tile.TileContext(nc) as tc, ExitStack() as ctx:
        p_const = ctx.enter_context(tc.tile_pool(name="const", bufs=1))
        p_xstage = ctx.enter_context(tc.tile_pool(name="xstage", bufs=2))
        p_xT = ctx.enter_context(tc.tile_pool(name="xT", bufs=2))
        p_w1 = ctx.enter_context(tc.tile_pool(name="w1", bufs=4))
        p_w2 = ctx.enter_context(tc.tile_pool(name="w2", bufs=2))
        p_h = ctx.enter_context(tc.tile_pool(name="hT", bufs=2))
        p_oe = ctx.enter_context(tc.tile_pool(name="oe", bufs=2))
        p_osh = ctx.enter_context(tc.tile_pool(name="osh", bufs=1))
        p_acc = ctx.enter_context(tc.tile_pool(name="acc", bufs=1))
        p_bias = ctx.enter_context(tc.tile_pool(name="bias", bufs=2))
        p_gw = ctx.enter_context(tc.tile_pool(name="gw", bufs=1))
        p_gt = ctx.enter_context(tc.tile_pool(name="gt", bufs=2))
        p_sm = ctx.enter_context(tc.tile_pool(name="sm", bufs=3))
        p_tmp = ctx.enter_context(tc.tile_pool(name="tmp", bufs=2))
        ps_h = ctx.enter_context(tc.tile_pool(name="psh", bufs=3, space="PSUM"))
        ps_o = ctx.enter_context(tc.tile_pool(name="pso", bufs=2, space="PSUM"))
        ps_t = ctx.enter_context(tc.tile_pool(name="pst", bufs=3, space="PSUM"))

        # Build identity/ones on-chip: no DMA ahead of the x transfers.
        ident_sb = p_const.tile([128, 128], F32)
        nc.gpsimd.memset(ident_sb, 0.0)
        nc.gpsimd.affine_select(
            out=ident_sb,
            in_=ident_sb,
            compare_op=ALU.not_equal,
            fill=1.0,
            base=0,
            pattern=[[-1, 128]],
            channel_multiplier=1,
        )
        identr_sb = p_const.tile([128, 128], mm_dt)
        nc.scalar.copy(out=identr_sb, in_=ident_sb)
        onesf_sb = p_const.tile([1, 128], F32)
        nc.gpsimd.memset(onesf_sb, 1.0)
        ones_sb = p_const.tile([1, 128], mm_dt)
        nc.scalar.copy(out=ones_sb, in_=onesf_sb)
        # PE warm-up: harmless matmuls on the identity while the first x/W
        # DMAs are in flight, so the HAM clock gate opens before real work.
        for _ in range(24):
            pw = ps_t.tile([128, 128], F32, tag="pt", name="pw")
            nc.tensor.matmul(pw, lhsT=identr_sb, rhs=identr_sb, start=True, stop=True)
        def transpose_x(x_dram):
            """[BL, D] natural -> xT tile [128, NKD, BL] (d on partitions).

            j-outer so slab j is complete (and consumable by L1/gates)
            after only NBT transposes; 4 transposes share one PSUM bank and
            drain with a single contiguous ACT copy.
            """
            xT = p_xT.tile([128, NKD, BL], mm_dt, tag="xT")
            xsts = x_dram
            for j in range(NKD):
                pt = ps_t.tile([128, BL], mm_dt, tag="pt")
                for bt in range(NBT):
                    nc.tensor.transpose(
                        pt[:, bt * 128 : (bt + 1) * 128],
                        xsts[bt][:, j * 128 : (j + 1) * 128],
                        identr_sb,
                    )
                nc.scalar.copy(out=xT[:, j, :], in_=pt)
            return xT

        def load_xstage(x_dram, bts=range(NBT)):
            xsts = []
            for bt in bts:
                xst = p_xstage.tile(
                    [128, D], mm_dt, tag=f"xst{bt}", name=f"xst{bt}", bufs=1
                )
                nc.sync.dma_start(
                    out=xst,
                    in_=x_dram[bt * 128 : (bt + 1) * 128, :].bitcast(mm_dt),
                )
                xsts.append(xst)
            return xsts

        def compute_gate(xT, wg_2d, bias_1d, K, tag):
            """softmax(x @ Wg + bg) -> gw tile [128, NBT, K] (b on partitions)."""
            wg_sb = p_sm.tile([128, NKD, K], mm_dt, tag=f"wg{K}")
            nc.sync.dma_start(
                out=wg_sb, in_=wg_2d.rearrange("(kt p) k -> p kt k", p=128)
            )
            bg_sb = p_sm.tile([K, 1], F32, tag=f"bg{K}")
            nc.sync.dma_start(
                out=bg_sb, in_=bias_1d.rearrange("(k one) -> k one", one=1)
            )
            pg = ps_t.tile([K, BL], F32, tag="pt")
            for kt in range(NKD):
                nc.tensor.matmul(
                    pg,
                    lhsT=wg_sb[:, kt, :],
                    rhs=xT[:, kt, :],
                    start=(kt == 0),
                    stop=(kt == NKD - 1),
                )
            glT = p_gt.tile([K, BL], F32, tag="glT")
            nc.scalar.activation(
                out=glT, in_=pg, func=AF.Identity, bias=bg_sb, scale=1.0
            )
            gw = p_gw.tile([128, NBT, K], F32, tag=tag)
            for bt in range(NBT):
                ptg = ps_t.tile([128, K], F32, tag="pt")
                nc.tensor.transpose(
                    ptg, glT[:, bt * 128 : (bt + 1) * 128], ident_sb[:K, :K]
                )
                nm = p_sm.tile([128, 1], F32, tag="nm")
                nc.vector.reduce_max(out=nm, in_=ptg, axis=AX.X, negate=True)
                esb = p_sm.tile([128, K], F32, tag="esb")
                nc.scalar.activation(
                    out=esb, in_=ptg, func=AF.Exp, bias=nm, scale=1.0
                )
                ssb = p_sm.tile([128, 1], F32, tag="ssb")
                nc.vector.reduce_sum(out=ssb, in_=esb, axis=AX.X)
                rsb = p_sm.tile([128, 1], F32, tag="rsb")
                nc.vector.reciprocal(out=rsb, in_=ssb)
                nc.vector.tensor_scalar_mul(gw[:, bt, :], esb, rsb)
            return gw

        def expert(xT, w1_2d, b1_1d, w2_2d, b2_1d, out_pool, tag, bias_mm=False):
            """Two-layer MLP: relu(relu(x@W1+b1)@W2+b2) -> [128, NBT, H2]."""
            b1_sb = p_bias.tile([128, NMH], F32, tag="b1")
            nc.sync.dma_start(
                out=b1_sb, in_=b1_1d.rearrange("(mt p) -> p mt", p=128)
            )
            b2_sb = p_bias.tile([1, H2], mm_dt, tag="b2")
            nc.sync.dma_start(
                out=b2_sb, in_=b2_1d.rearrange("(one o) -> one o", one=1)
            )
            if not bias_mm:
                bb = ps_t.tile([128, H2], F32, tag="pt", name="bb")
                nc.tensor.matmul(
                    bb, lhsT=ones_sb, rhs=b2_sb, start=True, stop=True
                )
                b2bc = p_tmp.tile([128, H2], F32, tag="b2bc", name="b2bc")
                nc.scalar.copy(out=b2bc, in_=bb)
            w1r = w1_2d.rearrange("(kt p) h -> p kt h", p=128)
            hT = p_h.tile([128, NMH, BL], mm_dt, tag="hT")
            for mt in range(NMH):
                w1_sb = p_w1.tile([128, NKD, 128], mm_dt, tag="w1")
                nc.sync.dma_start(
                    out=w1_sb, in_=w1r[:, :, mt * 128 : (mt + 1) * 128]
                )

                ph = ps_h.tile([128, BL], F32, tag="ph")
                for kt in range(NKD):
                    nc.tensor.matmul(
                        ph,
                        lhsT=w1_sb[:, kt, :],
                        rhs=xT[:, kt, :],
                        start=(kt == 0),
                        stop=(kt == NKD - 1),
                    )
                nc.scalar.activation(
                    out=hT[:, mt, :],
                    in_=ph,
                    func=AF.Relu,
                    bias=b1_sb[:, mt : mt + 1],
                    scale=1.0,
                )
            w2_sb = p_w2.tile([128, NKH, H2], mm_dt, tag="w2")
            w2r = w2_2d.rearrange("(kt p) o -> p kt o", p=128)
            for g in range(0, NKH, 2):
                nc.sync.dma_start(
                    out=w2_sb[:, g : g + 2, :], in_=w2r[:, g : g + 2, :]
                )
            oe = out_pool.tile([128, NBT, H2], F32, tag=tag)
            for bt in range(NBT):
                po = ps_o.tile([128, H2], F32, tag="po")
                po2 = po
                for kt in range(NKH):
                    nc.tensor.matmul(
                        po,
                        lhsT=hT[:, kt, bt * 128 : (bt + 1) * 128],
                        rhs=w2_sb[:, kt, :],
                        start=(kt == 0),
                        stop=(False if bias_mm else kt == NKH - 1),
                    )
                if bias_mm:
                    nc.tensor.matmul(
                        po2, lhsT=ones_sb, rhs=b2_sb, start=False, stop=True
                    )
                    nc.scalar.activation(out=oe[:, bt, :], in_=po2, func=AF.Relu)
                else:
                    nc.vector.tensor_tensor(oe[:, bt, :], po, b2bc, ALU.add)
                    nc.gpsimd.tensor_scalar_max(oe[:, bt, :], oe[:, bt, :], 0.0)
            return oe

        accs = [None] * 4

        def accumulate(acc_idx, oe, gw, col, first):
            acc = accs[acc_idx]
            for bt in range(NBT):
                if first:
                    nc.vector.tensor_scalar_mul(
                        acc[:, bt, :], oe[:, bt, :], gw[:, bt, col : col + 1]
                    )
                else:
                    nc.vector.scalar_tensor_tensor(
                        out=acc[:, bt, :],
                        in0=oe[:, bt, :],
                        scalar=gw[:, bt, col : col + 1],
                        in1=acc[:, bt, :],
                        op0=ALU.mult,
                        op1=ALU.add,
                    )

        # ---- shared phase: shared experts kept resident, shared gate ----
        xT_sh = transpose_x(load_xstage(xs[3]))
        gws = compute_gate(xT_sh, Wsg[:], bsg[:], TOTAL_E, tag="gws")
        osh = []
        xsts_next = None
        for j in range(NSH):
            o = expert(
                xT_sh, W1h[j], b1h[j], W2h[j], b2h[j], p_osh, tag=f"osh{j}"
            )
            osh.append(o)
            if j == 0:
                xsts_next = load_xstage(xs[0])
        accs[3] = p_acc.tile([128, NBT, H2], F32, tag="acc3", name="acc3")
        accumulate(3, osh[0], gws, E_SPEC + 0, first=True)
        accumulate(3, osh[1], gws, E_SPEC + 1, first=False)

        # ---- domain phases ----
        for d in range(DOM):
            xT_d = transpose_x(xsts_next)
            gw_d = compute_gate(xT_d, Wg[d], bg[d], GATE_K, tag=f"gw{d}")
            accs[d] = p_acc.tile(
                [128, NBT, H2], F32, tag=f"acc{d}", name=f"acc{d}"
            )
            accumulate(d, osh[0], gw_d, NES + 0, first=True)
            accumulate(d, osh[1], gw_d, NES + 1, first=False)
            for i in range(NES):
                e = d * NES + i
                oe = expert(
                    xT_d, W1s[e], b1s[e], W2s[e], b2s[e], p_oe, tag="oe",
                    bias_mm=(e == E_SPEC - 1),
                )
                if i == 0 and d < DOM - 1:
                    xsts_next = load_xstage(xs[d + 1])
                accumulate(d, oe, gw_d, i, first=False)
                accumulate(3, oe, gws, e, first=False)
            yr = ys[d][:].rearrange("(bt p) o -> bt p o", p=128)
            for bt in range(NBT):
                nc.sync.dma_start(out=yr[bt], in_=accs[d][:, bt, :])
        yr3 = ys[3][:].rearrange("(bt p) o -> bt p o", p=128)
        for bt in range(NBT):
            nc.sync.dma_start(out=yr3[bt], in_=accs[3][:, bt, :])

    nc.compile()
    return nc


_NC_CACHE = {}


def _get_nc(mm_dt=F32R):
    key = str(mm_dt)
    if key not in _NC_CACHE:
        _NC_CACHE[key] = _build_nc(mm_dt)
    return _NC_CACHE[key]


def kernel(**inputs):
    return run_kernel(inputs)


def run_kernel(inputs, mm_dt=F32R, trace=False):
    nc = _get_nc(mm_dt)
    shard_names = ("x0", "x1", "x2", "x_shared")
    full = {k: np.ascontiguousarray(np.asarray(v, dtype=np.float32)) for k, v in inputs.items()}
    in_maps = []
    for c in range(N_CORES):
        m = {}
        for k, v in full.items():
            if k in shard_names:
                m[k] = v[c * BL : (c + 1) * BL]
            else:
                m[k] = v
        in_maps.append(m)
    res = run_bass_kernel_spmd(nc, in_maps, list(range(N_CORES)), trace=trace)
    outs = []
    for name in ("y0", "y1", "y2", "ysh"):
        outs.append(
            np.concatenate([res.results[c][name] for c in range(N_CORES)], axis=0)
        )
    out = tuple(outs)
    if trace:
        return out, res
    return out
